# revision 1
# baseline (speedup 1.0000x reference)
"""Batched int8 GEMM with scaling for TRN2: out[b] = round(alpha * (a[b] @ b[b]^T)).

Shapes (hardcoded per the problem spec): a [64,1024,128] int8, b [64,1024,128] int8,
alpha fp32 scalar -> out [64,1024,1024] int32.

Strategy:
- Shard batch dim B=64 across 8 NeuronCores (8 batches/core), no communication.
- Host-side prep: transpose to a^T [B,K,M] / b^T [B,K,N] (K=128 on partitions, the
  layout the PE array needs for both operands) and cast int8 -> bf16, which is exact
  for [-128,127]. Products (<=2^14) and K=128-deep sums (<=2^21) are exact in the
  fp32 PSUM accumulator, so the GEMM is bit-exact.
- Per (m,n) tile: one 128x128x512 matmul, then a single fused epilogue op
  (mul-by-alpha + fp32->int cast; HW cast is round-to-nearest-even, matching
  jnp.round) alternating between VectorE and ScalarE.
- Device output is int16 when alpha bounds |out| < 32768 (always true for the spec's
  alpha=2^-7: |acc| <= 128*128*128 = 2^21 -> |out| <= 16384), halving the dominant
  HBM write traffic; host upcasts to int32.
"""

import sys

sys.path.insert(0, "/opt/trn_rl_repo")

from contextlib import ExitStack

import ml_dtypes
import numpy as np

import concourse.tile as tile
from concourse import bacc, mybir
from concourse.bass_utils import run_bass_kernel_spmd

B, M, N, K = 64, 1024, 1024, 128
N_CORES = 8
BPC = B // N_CORES  # batches per core
MT = 128  # m-tile (PSUM partition dim)
NT = 512  # n-tile (one PSUM bank of fp32)

ACC_MAX = 128 * 128 * K  # max |a@b^T| entry for int8 operands

_cache: dict = {}


def _build(alpha: float, out16: bool):
    out_dt = mybir.dt.int16 if out16 else mybir.dt.int32
    nc = bacc.Bacc(
        "TRN2", target_bir_lowering=False, debug=False, num_devices=N_CORES
    )
    aT = nc.dram_tensor("aT", [BPC, K, M], mybir.dt.bfloat16, kind="ExternalInput").ap()
    bT = nc.dram_tensor("bT", [BPC, K, N], mybir.dt.bfloat16, kind="ExternalInput").ap()
    out = nc.dram_tensor("out", [BPC, M, N], out_dt, kind="ExternalOutput").ap()

    with tile.TileContext(nc) as tc, ExitStack() as ctx:
        a_pool = ctx.enter_context(tc.tile_pool(name="a", bufs=2))
        b_pool = ctx.enter_context(tc.tile_pool(name="b", bufs=2))
        ps_pool = ctx.enter_context(tc.tile_pool(name="ps", bufs=6, space="PSUM"))
        o_pool = ctx.enter_context(tc.tile_pool(name="o", bufs=6))

        tile_idx = 0
        for i in range(BPC):
            at = a_pool.tile([K, M], mybir.dt.bfloat16)
            nc.sync.dma_start(at[:], aT[i])
            bt = b_pool.tile([K, N], mybir.dt.bfloat16)
            nc.sync.dma_start(bt[:], bT[i])
            for m in range(M // MT):
                ot = o_pool.tile([MT, N], out_dt)
                for n in range(N // NT):
                    ps = ps_pool.tile([MT, NT], mybir.dt.float32)
                    nc.tensor.matmul(
                        ps[:],
                        at[:, m * MT : (m + 1) * MT],
                        bt[:, n * NT : (n + 1) * NT],
                        start=True,
                        stop=True,
                    )
                    osl = ot[:, n * NT : (n + 1) * NT]
                    # VectorE is ~2x faster than ScalarE here; split 2:1
                    if tile_idx % 3 < 2:
                        nc.vector.tensor_scalar_mul(osl, ps[:], alpha)
                    else:
                        nc.scalar.mul(osl, ps[:], alpha)
                    tile_idx += 1
                nc.sync.dma_start(out[i, m * MT : (m + 1) * MT, :], ot[:])

    nc.compile()
    return nc


def _get(alpha: float, out16: bool):
    key = (alpha, out16)
    if key not in _cache:
        _cache[key] = _build(alpha, out16)
    return _cache[key]


def kernel(a: np.ndarray, b: np.ndarray, alpha: np.ndarray) -> np.ndarray:
    alpha_f = float(np.asarray(alpha))
    out16 = abs(alpha_f) * ACC_MAX < 32767.5

    aT = np.ascontiguousarray(a.transpose(0, 2, 1)).astype(ml_dtypes.bfloat16)
    bT = np.ascontiguousarray(b.transpose(0, 2, 1)).astype(ml_dtypes.bfloat16)

    nc = _get(alpha_f, out16)
    in_maps = [
        {"aT": aT[c * BPC : (c + 1) * BPC], "bT": bT[c * BPC : (c + 1) * BPC]}
        for c in range(N_CORES)
    ]
    res = run_bass_kernel_spmd(nc, in_maps, list(range(N_CORES))).results
    out = np.concatenate([res[c]["out"] for c in range(N_CORES)], axis=0)
    return out.astype(np.int32)


# revision 2
# speedup vs baseline: 1.0213x; 1.0213x over previous
"""Batched int8 GEMM with scaling for TRN2: out[b] = round(alpha * (a[b] @ b[b]^T)).

Shapes (hardcoded per the problem spec): a [64,1024,128] int8, b [64,1024,128] int8,
alpha fp32 scalar -> out [64,1024,1024] int32.

Strategy:
- Shard batch dim B=64 across 8 NeuronCores (8 batches/core), no communication.
- Host-side prep: transpose to a^T [B,K,M] / b^T [B,K,N] (K=128 on partitions, the
  layout the PE array needs for both operands) and cast int8 -> bf16, which is exact
  for [-128,127]. Products (<=2^14) and K=128-deep sums (<=2^21) are exact in the
  fp32 PSUM accumulator, so the GEMM is bit-exact.
- Per (m,n) tile: one 128x128x512 matmul, then a single fused epilogue op
  (mul-by-alpha + fp32->int cast; HW cast is round-to-nearest-even, matching
  jnp.round) alternating between VectorE and ScalarE.
- Device output is int16 when alpha bounds |out| < 32768 (always true for the spec's
  alpha=2^-7: |acc| <= 128*128*128 = 2^21 -> |out| <= 16384), halving the dominant
  HBM write traffic; host upcasts to int32.
"""

import sys

sys.path.insert(0, "/opt/trn_rl_repo")

from contextlib import ExitStack

import ml_dtypes
import numpy as np

import concourse.tile as tile
from concourse import bacc, mybir
from concourse.bass_utils import run_bass_kernel_spmd

B, M, N, K = 64, 1024, 1024, 128
N_CORES = 8
BPC = B // N_CORES  # batches per core
MT = 128  # m-tile (PSUM partition dim)
NT = 512  # n-tile (one PSUM bank of fp32)

ACC_MAX = 128 * 128 * K  # max |a@b^T| entry for int8 operands

_cache: dict = {}


def _build(alpha: float, out16: bool):
    out_dt = mybir.dt.int16 if out16 else mybir.dt.int32
    nc = bacc.Bacc(
        "TRN2", target_bir_lowering=False, debug=False, num_devices=N_CORES
    )
    aT = nc.dram_tensor("aT", [BPC, K, M], mybir.dt.bfloat16, kind="ExternalInput").ap()
    bT = nc.dram_tensor("bT", [BPC, K, N], mybir.dt.bfloat16, kind="ExternalInput").ap()
    out = nc.dram_tensor("out", [BPC, M, N], out_dt, kind="ExternalOutput").ap()

    # out[i] viewed as [p, m, n] so a whole batch is one 2 MB DMA
    out_r = out.rearrange("i (m p) n -> i p m n", p=MT)

    with tile.TileContext(nc) as tc, ExitStack() as ctx:
        a_pool = ctx.enter_context(tc.tile_pool(name="a", bufs=3))
        b_pool = ctx.enter_context(tc.tile_pool(name="b", bufs=3))
        ps_pool = ctx.enter_context(tc.tile_pool(name="ps", bufs=4, space="PSUM"))
        o_pool = ctx.enter_context(tc.tile_pool(name="o", bufs=2))

        for i in range(BPC):
            # inputs on the ACT HWDGE ring; outputs on the SP ring
            at = a_pool.tile([K, M], mybir.dt.bfloat16)
            nc.scalar.dma_start(at[:], aT[i])
            bt = b_pool.tile([K, N], mybir.dt.bfloat16)
            nc.scalar.dma_start(bt[:], bT[i])
            ot = o_pool.tile([MT, M // MT, N], out_dt)
            for m in range(M // MT):
                ps = ps_pool.tile([MT, N], mybir.dt.float32)
                for n in range(N // NT):
                    nc.tensor.matmul(
                        ps[:, n * NT : (n + 1) * NT],
                        at[:, m * MT : (m + 1) * MT],
                        bt[:, n * NT : (n + 1) * NT],
                        start=True,
                        stop=True,
                    )
                # epilogue split between VectorE and ScalarE
                nc.vector.tensor_scalar_mul(ot[:, m, :NT], ps[:, :NT], alpha)
                nc.scalar.mul(ot[:, m, NT:], ps[:, NT:], alpha)
            nc.sync.dma_start(out_r[i], ot[:])

    nc.compile()
    return nc


def _get(alpha: float, out16: bool):
    key = (alpha, out16)
    if key not in _cache:
        _cache[key] = _build(alpha, out16)
    return _cache[key]


def kernel(a: np.ndarray, b: np.ndarray, alpha: np.ndarray) -> np.ndarray:
    alpha_f = float(np.asarray(alpha))
    out16 = abs(alpha_f) * ACC_MAX < 32767.5

    aT = np.ascontiguousarray(a.transpose(0, 2, 1)).astype(ml_dtypes.bfloat16)
    bT = np.ascontiguousarray(b.transpose(0, 2, 1)).astype(ml_dtypes.bfloat16)

    nc = _get(alpha_f, out16)
    in_maps = [
        {"aT": aT[c * BPC : (c + 1) * BPC], "bT": bT[c * BPC : (c + 1) * BPC]}
        for c in range(N_CORES)
    ]
    res = run_bass_kernel_spmd(nc, in_maps, list(range(N_CORES))).results
    out = np.concatenate([res[c]["out"] for c in range(N_CORES)], axis=0)
    return out.astype(np.int32)


# revision 3
# speedup vs baseline: 1.0556x; 1.0336x over previous
"""Batched int8 GEMM with scaling for TRN2: out[b] = round(alpha * (a[b] @ b[b]^T)).

Shapes (hardcoded per the problem spec): a [64,1024,128] int8, b [64,1024,128] int8,
alpha fp32 scalar -> out [64,1024,1024] int32.

Strategy:
- Shard batch dim B=64 across 8 NeuronCores (8 batches/core), no communication.
- Host-side prep: transpose to a^T [B,K,M] / b^T [B,K,N] (K=128 on partitions, the
  layout the PE array needs for both operands) and cast int8 -> bf16, which is exact
  for [-128,127]. Products (<=2^14) and K=128-deep sums (<=2^21) are exact in the
  fp32 PSUM accumulator, so the GEMM is bit-exact.
- Per (m,n) tile: one 128x128x512 matmul, then a single fused epilogue op
  (mul-by-alpha + fp32->int cast; HW cast is round-to-nearest-even, matching
  jnp.round) alternating between VectorE and ScalarE.
- Device output is int16 when alpha bounds |out| < 32768 (always true for the spec's
  alpha=2^-7: |acc| <= 128*128*128 = 2^21 -> |out| <= 16384), halving the dominant
  HBM write traffic; host upcasts to int32.
"""

import sys

sys.path.insert(0, "/opt/trn_rl_repo")

from contextlib import ExitStack

import ml_dtypes
import numpy as np

import concourse.tile as tile
from concourse import bacc, mybir
from concourse.bass_utils import run_bass_kernel_spmd

B, M, N, K = 64, 1024, 1024, 128
N_CORES = 8
BPC = B // N_CORES  # batches per core
MT = 128  # m-tile (PSUM partition dim)
NT = 512  # n-tile (one PSUM bank of fp32)

ACC_MAX = 128 * 128 * K  # max |a@b^T| entry for int8 operands

_cache: dict = {}


def _build(alpha: float, out16: bool):
    out_dt = mybir.dt.int16 if out16 else mybir.dt.int32
    nc = bacc.Bacc(
        "TRN2", target_bir_lowering=False, debug=False, num_devices=N_CORES
    )
    aT = nc.dram_tensor("aT", [BPC, K, M], mybir.dt.bfloat16, kind="ExternalInput").ap()
    bT = nc.dram_tensor("bT", [BPC, K, N], mybir.dt.bfloat16, kind="ExternalInput").ap()
    out = nc.dram_tensor("out", [BPC, M, N], out_dt, kind="ExternalOutput").ap()

    # out[i] viewed as [p, m, n] so a whole batch is one 2 MB DMA
    out_r = out.rearrange("i (m p) n -> i p m n", p=MT)

    with tile.TileContext(nc) as tc, ExitStack() as ctx:
        a_pool = ctx.enter_context(tc.tile_pool(name="a", bufs=3))
        b_pool = ctx.enter_context(tc.tile_pool(name="b", bufs=3))
        ps_pool = ctx.enter_context(tc.tile_pool(name="ps", bufs=8, space="PSUM"))
        o_pool = ctx.enter_context(tc.tile_pool(name="o", bufs=3))

        MH = M // MT // 2  # m-tiles per output DMA (half batch = 1 MB)
        for i in range(BPC):
            # inputs on the ACT HWDGE ring; outputs on the SP ring
            at = a_pool.tile([K, M], mybir.dt.bfloat16)
            nc.scalar.dma_start(at[:], aT[i])
            bt = b_pool.tile([K, N], mybir.dt.bfloat16)
            nc.scalar.dma_start(bt[:], bT[i])
            ot = o_pool.tile([MT, M // MT, N], out_dt)
            for m in range(M // MT):
                for n in range(N // NT):
                    ps = ps_pool.tile([MT, NT], mybir.dt.float32)
                    nc.tensor.matmul(
                        ps[:],
                        at[:, m * MT : (m + 1) * MT],
                        bt[:, n * NT : (n + 1) * NT],
                        start=True,
                        stop=True,
                    )
                    # epilogue split between VectorE and ScalarE,
                    # decoupled per PSUM bank
                    osl = ot[:, m, n * NT : (n + 1) * NT]
                    if n == 0:
                        nc.vector.tensor_scalar_mul(osl, ps[:], alpha)
                    else:
                        nc.scalar.mul(osl, ps[:], alpha)
                if m % MH == MH - 1:
                    h = m // MH
                    nc.sync.dma_start(
                        out_r[i][:, h * MH : (h + 1) * MH],
                        ot[:, h * MH : (h + 1) * MH],
                    )

    nc.compile()
    return nc


def _get(alpha: float, out16: bool):
    key = (alpha, out16)
    if key not in _cache:
        _cache[key] = _build(alpha, out16)
    return _cache[key]


def kernel(a: np.ndarray, b: np.ndarray, alpha: np.ndarray) -> np.ndarray:
    alpha_f = float(np.asarray(alpha))
    out16 = abs(alpha_f) * ACC_MAX < 32767.5

    aT = np.ascontiguousarray(a.transpose(0, 2, 1)).astype(ml_dtypes.bfloat16)
    bT = np.ascontiguousarray(b.transpose(0, 2, 1)).astype(ml_dtypes.bfloat16)

    nc = _get(alpha_f, out16)
    in_maps = [
        {"aT": aT[c * BPC : (c + 1) * BPC], "bT": bT[c * BPC : (c + 1) * BPC]}
        for c in range(N_CORES)
    ]
    res = run_bass_kernel_spmd(nc, in_maps, list(range(N_CORES))).results
    out = np.concatenate([res[c]["out"] for c in range(N_CORES)], axis=0)
    return out.astype(np.int32)


# revision 4
# speedup vs baseline: 1.0747x; 1.0181x over previous
"""Batched int8 GEMM with scaling for TRN2: out[b] = round(alpha * (a[b] @ b[b]^T)).

Shapes (hardcoded per the problem spec): a [64,1024,128] int8, b [64,1024,128] int8,
alpha fp32 scalar -> out [64,1024,1024] int32.

Strategy:
- Shard batch dim B=64 across 8 NeuronCores (8 batches/core), no communication.
- Host-side prep: transpose to a^T [B,K,M] / b^T [B,K,N] (K=128 on partitions, the
  layout the PE array needs for both operands) and cast int8 -> bf16, which is exact
  for [-128,127]. Products (<=2^14) and K=128-deep sums (<=2^21) are exact in the
  fp32 PSUM accumulator, so the GEMM is bit-exact.
- Per (m,n) tile: one 128x128x512 matmul, then a single fused epilogue op
  (mul-by-alpha + fp32->int cast; HW cast is round-to-nearest-even, matching
  jnp.round) alternating between VectorE and ScalarE.
- Device output is int16 when alpha bounds |out| < 32768 (always true for the spec's
  alpha=2^-7: |acc| <= 128*128*128 = 2^21 -> |out| <= 16384), halving the dominant
  HBM write traffic; host upcasts to int32.
"""

import sys

sys.path.insert(0, "/opt/trn_rl_repo")

from contextlib import ExitStack

import ml_dtypes
import numpy as np

import concourse.tile as tile
from concourse import bacc, mybir
from concourse.bass_utils import run_bass_kernel_spmd

B, M, N, K = 64, 1024, 1024, 128
N_CORES = 8
BPC = B // N_CORES  # batches per core
MT = 128  # m-tile (PSUM partition dim)
NT = 512  # n-tile (one PSUM bank of fp32)

ACC_MAX = 128 * 128 * K  # max |a@b^T| entry for int8 operands

_cache: dict = {}


def _build(alpha: float, out16: bool):
    out_dt = mybir.dt.int16 if out16 else mybir.dt.int32
    nc = bacc.Bacc(
        "TRN2", target_bir_lowering=False, debug=False, num_devices=N_CORES
    )
    aT = nc.dram_tensor("aT", [BPC, K, M], mybir.dt.bfloat16, kind="ExternalInput").ap()
    bT = nc.dram_tensor("bT", [BPC, K, N], mybir.dt.bfloat16, kind="ExternalInput").ap()
    out = nc.dram_tensor("out", [BPC, M, N], out_dt, kind="ExternalOutput").ap()

    # out[i] viewed as [p, m, n] so a whole batch is one 2 MB DMA
    out_r = out.rearrange("i (m p) n -> i p m n", p=MT)

    with tile.TileContext(nc) as tc, ExitStack() as ctx:
        a_pool = ctx.enter_context(tc.tile_pool(name="a", bufs=3))
        b_pool = ctx.enter_context(tc.tile_pool(name="b", bufs=3))
        ps_pool = ctx.enter_context(tc.tile_pool(name="ps", bufs=8, space="PSUM"))
        o_pool = ctx.enter_context(tc.tile_pool(name="o", bufs=3))

        MH = M // MT // 2  # m-tiles per output DMA (half batch = 1 MB)
        for i in range(BPC):
            # inputs via SWDGE on idle GpSimd (own descriptor stream, doesn't
            # steal ScalarE time or queue behind its epilogue ops);
            # outputs on the SP HWDGE ring
            at = a_pool.tile([K, M], mybir.dt.bfloat16)
            nc.gpsimd.dma_start(at[:], aT[i])
            bt = b_pool.tile([K, N], mybir.dt.bfloat16)
            nc.gpsimd.dma_start(bt[:], bT[i])
            ot = o_pool.tile([MT, M // MT, N], out_dt)
            for m in range(M // MT):
                for n in range(N // NT):
                    ps = ps_pool.tile([MT, NT], mybir.dt.float32)
                    nc.tensor.matmul(
                        ps[:],
                        at[:, m * MT : (m + 1) * MT],
                        bt[:, n * NT : (n + 1) * NT],
                        start=True,
                        stop=True,
                    )
                    # epilogue split between VectorE and ScalarE,
                    # decoupled per PSUM bank
                    osl = ot[:, m, n * NT : (n + 1) * NT]
                    if n == 0:
                        nc.vector.tensor_scalar_mul(osl, ps[:], alpha)
                    else:
                        nc.scalar.mul(osl, ps[:], alpha)
                if m % MH == MH - 1:
                    h = m // MH
                    nc.sync.dma_start(
                        out_r[i][:, h * MH : (h + 1) * MH],
                        ot[:, h * MH : (h + 1) * MH],
                    )

    nc.compile()
    return nc


def _get(alpha: float, out16: bool):
    key = (alpha, out16)
    if key not in _cache:
        _cache[key] = _build(alpha, out16)
    return _cache[key]


def kernel(a: np.ndarray, b: np.ndarray, alpha: np.ndarray) -> np.ndarray:
    alpha_f = float(np.asarray(alpha))
    out16 = abs(alpha_f) * ACC_MAX < 32767.5

    aT = np.ascontiguousarray(a.transpose(0, 2, 1)).astype(ml_dtypes.bfloat16)
    bT = np.ascontiguousarray(b.transpose(0, 2, 1)).astype(ml_dtypes.bfloat16)

    nc = _get(alpha_f, out16)
    in_maps = [
        {"aT": aT[c * BPC : (c + 1) * BPC], "bT": bT[c * BPC : (c + 1) * BPC]}
        for c in range(N_CORES)
    ]
    res = run_bass_kernel_spmd(nc, in_maps, list(range(N_CORES))).results
    out = np.concatenate([res[c]["out"] for c in range(N_CORES)], axis=0)
    return out.astype(np.int32)


# revision 6
# speedup vs baseline: 1.0883x; 1.0127x over previous
"""Batched int8 GEMM with scaling for TRN2: out[b] = round(alpha * (a[b] @ b[b]^T)).

Shapes (hardcoded per the problem spec): a [64,1024,128] int8, b [64,1024,128] int8,
alpha fp32 scalar -> out [64,1024,1024] int32.

Strategy:
- Shard batch dim B=64 across 8 NeuronCores (8 batches/core), no communication.
- Host-side prep: transpose to a^T [B,K,M] / b^T [B,K,N] (K=128 on partitions, the
  layout the PE array needs for both operands) and cast int8 -> bf16, which is exact
  for [-128,127]. Products (<=2^14) and K=128-deep sums (<=2^21) are exact in the
  fp32 PSUM accumulator, so the GEMM is bit-exact.
- Per (m,n) tile: one 128x128x512 matmul, then a single fused epilogue op
  (mul-by-alpha + fp32->int cast; HW cast is round-to-nearest-even, matching
  jnp.round) alternating between VectorE and ScalarE.
- Device output is int16 when alpha bounds |out| < 32768 (always true for the spec's
  alpha=2^-7: |acc| <= 128*128*128 = 2^21 -> |out| <= 16384), halving the dominant
  HBM write traffic; host upcasts to int32.
"""

import sys

sys.path.insert(0, "/opt/trn_rl_repo")

from contextlib import ExitStack

import ml_dtypes
import numpy as np

import concourse.tile as tile
from concourse import bacc, mybir
from concourse.bass_utils import run_bass_kernel_spmd

B, M, N, K = 64, 1024, 1024, 128
N_CORES = 8
BPC = B // N_CORES  # batches per core
MT = 128  # m-tile (PSUM partition dim)
NT = 512  # n-tile (one PSUM bank of fp32)

ACC_MAX = 128 * 128 * K  # max |a@b^T| entry for int8 operands

_cache: dict = {}


def _build(alpha: float, out16: bool):
    out_dt = mybir.dt.int16 if out16 else mybir.dt.int32
    nc = bacc.Bacc(
        "TRN2", target_bir_lowering=False, debug=False, num_devices=N_CORES
    )
    aT = nc.dram_tensor("aT", [BPC, K, M], mybir.dt.bfloat16, kind="ExternalInput").ap()
    bT = nc.dram_tensor("bT", [BPC, K, N], mybir.dt.bfloat16, kind="ExternalInput").ap()
    out = nc.dram_tensor("out", [BPC, M, N], out_dt, kind="ExternalOutput").ap()

    # out[i] viewed as [p, m, n] so a whole batch is one 2 MB DMA
    out_r = out.rearrange("i (m p) n -> i p m n", p=MT)

    with tile.TileContext(nc) as tc, ExitStack() as ctx:
        a_pool = ctx.enter_context(tc.tile_pool(name="a", bufs=4))
        b_pool = ctx.enter_context(tc.tile_pool(name="b", bufs=4))
        ps_pool = ctx.enter_context(tc.tile_pool(name="ps", bufs=4, space="PSUM"))
        o_pool = ctx.enter_context(tc.tile_pool(name="o", bufs=4))

        MH = M // MT // 2  # m-tiles per output DMA (half batch = 1 MB)
        for i in range(BPC):
            # inputs via SWDGE on idle GpSimd (own descriptor stream, doesn't
            # steal ScalarE time or queue behind its epilogue ops);
            # outputs on the SP HWDGE ring
            at = a_pool.tile([K, M], mybir.dt.bfloat16)
            nc.gpsimd.dma_start(at[:], aT[i])
            bt = b_pool.tile([K, N], mybir.dt.bfloat16)
            nc.gpsimd.dma_start(bt[:], bT[i])
            ot = o_pool.tile([MT, M // MT, N], out_dt)
            for m in range(M // MT):
                ps = ps_pool.tile([MT, N], mybir.dt.float32)
                for n in range(N // NT):
                    nc.tensor.matmul(
                        ps[:, n * NT : (n + 1) * NT],
                        at[:, m * MT : (m + 1) * MT],
                        bt[:, n * NT : (n + 1) * NT],
                        start=True,
                        stop=True,
                    )
                # one fused epilogue op per m-tile, alternating engines
                osl = ot[:, m, :]
                if m % 2 == 0:
                    nc.vector.tensor_scalar_mul(osl, ps[:], alpha)
                else:
                    nc.scalar.mul(osl, ps[:], alpha)
                if m % MH == MH - 1:
                    # output halves alternate between the two HWDGE rings
                    h = m // MH
                    eng = nc.sync if (i * 2 + h) % 2 == 0 else nc.scalar
                    eng.dma_start(
                        out_r[i][:, h * MH : (h + 1) * MH],
                        ot[:, h * MH : (h + 1) * MH],
                    )

    nc.compile()
    return nc


def _get(alpha: float, out16: bool):
    key = (alpha, out16)
    if key not in _cache:
        _cache[key] = _build(alpha, out16)
    return _cache[key]


def kernel(a: np.ndarray, b: np.ndarray, alpha: np.ndarray) -> np.ndarray:
    alpha_f = float(np.asarray(alpha))
    out16 = abs(alpha_f) * ACC_MAX < 32767.5

    aT = np.ascontiguousarray(a.transpose(0, 2, 1)).astype(ml_dtypes.bfloat16)
    bT = np.ascontiguousarray(b.transpose(0, 2, 1)).astype(ml_dtypes.bfloat16)

    nc = _get(alpha_f, out16)
    in_maps = [
        {"aT": aT[c * BPC : (c + 1) * BPC], "bT": bT[c * BPC : (c + 1) * BPC]}
        for c in range(N_CORES)
    ]
    res = run_bass_kernel_spmd(nc, in_maps, list(range(N_CORES))).results
    out = np.concatenate([res[c]["out"] for c in range(N_CORES)], axis=0)
    return out.astype(np.int32)


# revision 7
# speedup vs baseline: 1.1442x; 1.0513x over previous
"""Batched int8 GEMM with scaling for TRN2: out[b] = round(alpha * (a[b] @ b[b]^T)).

Shapes (hardcoded per the problem spec): a [64,1024,128] int8, b [64,1024,128] int8,
alpha fp32 scalar -> out [64,1024,1024] int32.

Strategy:
- Shard batch dim B=64 across 8 NeuronCores (8 batches/core), no communication.
- Host-side prep: transpose to a^T [B,K,M] / b^T [B,K,N] (K=128 on partitions, the
  layout the PE array needs for both operands) and cast int8 -> bf16, which is exact
  for [-128,127]. Products (<=2^14) and K=128-deep sums (<=2^21) are exact in the
  fp32 PSUM accumulator, so the GEMM is bit-exact.
- Per (m,n) tile: one 128x128x512 matmul, then a single fused epilogue op
  (mul-by-alpha + fp32->int cast; HW cast is round-to-nearest-even, matching
  jnp.round) alternating between VectorE and ScalarE.
- Device output is int16 when alpha bounds |out| < 32768 (always true for the spec's
  alpha=2^-7: |acc| <= 128*128*128 = 2^21 -> |out| <= 16384), halving the dominant
  HBM write traffic; host upcasts to int32.
"""

import sys

sys.path.insert(0, "/opt/trn_rl_repo")

from contextlib import ExitStack

import ml_dtypes
import numpy as np

import concourse.tile as tile
from concourse import bacc, mybir
from concourse.bass_utils import run_bass_kernel_spmd

B, M, N, K = 64, 1024, 1024, 128
N_CORES = 8
BPC = B // N_CORES  # batches per core
MT = 128  # m-tile (PSUM partition dim)
NT = 512  # n-tile (one PSUM bank of fp32)

ACC_MAX = 128 * 128 * K  # max |a@b^T| entry for int8 operands

_cache: dict = {}


def _build(alpha: float, out16: bool):
    out_dt = mybir.dt.int16 if out16 else mybir.dt.int32
    nc = bacc.Bacc(
        "TRN2", target_bir_lowering=False, debug=False, num_devices=N_CORES
    )
    aT = nc.dram_tensor("aT", [BPC, K, M], mybir.dt.bfloat16, kind="ExternalInput").ap()
    bT = nc.dram_tensor("bT", [BPC, K, N], mybir.dt.bfloat16, kind="ExternalInput").ap()
    out = nc.dram_tensor("out", [BPC, M, N], out_dt, kind="ExternalOutput").ap()

    # out[i] viewed as [p, m, n] so a whole batch is one 2 MB DMA
    out_r = out.rearrange("i (m p) n -> i p m n", p=MT)

    with tile.TileContext(nc) as tc, ExitStack() as ctx:
        a_pool = ctx.enter_context(tc.tile_pool(name="a", bufs=4))
        b_pool = ctx.enter_context(tc.tile_pool(name="b", bufs=4))
        ps_pool = ctx.enter_context(tc.tile_pool(name="ps", bufs=4, space="PSUM"))
        o_pool = ctx.enter_context(tc.tile_pool(name="o", bufs=8))

        MH = M // MT // 2  # m-tiles per output DMA (half batch = 1 MB)
        for i in range(BPC):
            # steady-state inputs via SWDGE on idle GpSimd (own descriptor
            # stream, doesn't steal ScalarE time or queue behind its epilogue
            # ops). First batches go via the idle-at-start sync engine to cut
            # pipeline fill.
            in_eng = nc.sync if i < 2 else nc.gpsimd
            at = a_pool.tile([K, M], mybir.dt.bfloat16)
            in_eng.dma_start(at[:], aT[i])
            bt = b_pool.tile([K, N], mybir.dt.bfloat16)
            in_eng.dma_start(bt[:], bT[i])
            for h in range(2):
                ot = o_pool.tile([MT, MH, N], out_dt)
                for mh in range(MH):
                    m = h * MH + mh
                    ps = ps_pool.tile([MT, N], mybir.dt.float32)
                    for n in range(N // NT):
                        nc.tensor.matmul(
                            ps[:, n * NT : (n + 1) * NT],
                            at[:, m * MT : (m + 1) * MT],
                            bt[:, n * NT : (n + 1) * NT],
                            start=True,
                            stop=True,
                        )
                    # one fused epilogue op per m-tile, alternating engines
                    osl = ot[:, mh, :]
                    if m % 2 == 0:
                        nc.vector.tensor_scalar_mul(osl, ps[:], alpha)
                    else:
                        nc.scalar.mul(osl, ps[:], alpha)
                dst = out_r[i][:, h * MH : (h + 1) * MH]
                if i < BPC - 1:
                    # halves alternate between the two HWDGE rings
                    eng = nc.sync if (i * 2 + h) % 2 == 0 else nc.scalar
                    eng.dma_start(dst, ot[:])
                else:
                    # last batch: quarter-DMAs on both rings to cut the tail
                    MQ = MH // 2
                    for q in range(2):
                        eng = nc.sync if q % 2 == 0 else nc.scalar
                        eng.dma_start(
                            dst[:, q * MQ : (q + 1) * MQ],
                            ot[:, q * MQ : (q + 1) * MQ],
                        )

    nc.compile()
    return nc


def _get(alpha: float, out16: bool):
    key = (alpha, out16)
    if key not in _cache:
        _cache[key] = _build(alpha, out16)
    return _cache[key]


def kernel(a: np.ndarray, b: np.ndarray, alpha: np.ndarray) -> np.ndarray:
    alpha_f = float(np.asarray(alpha))
    out16 = abs(alpha_f) * ACC_MAX < 32767.5

    aT = np.ascontiguousarray(a.transpose(0, 2, 1)).astype(ml_dtypes.bfloat16)
    bT = np.ascontiguousarray(b.transpose(0, 2, 1)).astype(ml_dtypes.bfloat16)

    nc = _get(alpha_f, out16)
    in_maps = [
        {"aT": aT[c * BPC : (c + 1) * BPC], "bT": bT[c * BPC : (c + 1) * BPC]}
        for c in range(N_CORES)
    ]
    res = run_bass_kernel_spmd(nc, in_maps, list(range(N_CORES))).results
    out = np.concatenate([res[c]["out"] for c in range(N_CORES)], axis=0)
    return out.astype(np.int32)
